# revision 30
# baseline (speedup 1.0000x reference)
"""Convpass adapter kernel for Trainium2, 8 NeuronCores, data-parallel over batch.

Computation (per image, N=1024 patches = 32x32 grid, C=768, dim=8):
    d1 = x @ Wd + bd                  # [N, 8]
    a1 = quick_gelu(d1)               # quick_gelu(v) = v*sigmoid(1.702v) = silu(1.702v)/1.702
    c2 = conv3x3(a1, Wc) + bc         # SAME padding on 32x32 grid
    a2 = quick_gelu(c2)
    out = a2 @ Wu + bu                # [N, 768]

Sharding: batch 64 -> 8 images per core. The problem is HBM-bandwidth bound
(~210 MFLOP vs 48 MiB of f32 I/O per core), so all device I/O is fp16: the
host quantizes x and upcasts the output (adds ~7e-4 rel err vs the 2e-2
budget) and HBM traffic halves to ~24.6 MiB/core.

DMA plan: all bulk traffic rides the sync HWDGE ring in strict FIFO order --
image loads first (batched 2 images = one 24 KiB line per partition; image 0
split into halves so the first matmul starts one half-load sooner), then
output stores backfill the ring as results appear. Stores are quarter-image
pieces (one contiguous 3 KiB line per partition) so the ring never idles
waiting for a full image. The tiny const blob rides the otherwise-idle scalar
ring: ALL weights/biases are packed into ONE [128, 1972] fp16 tensor (f32
biases as bitcast fp16 pairs) -> one descriptor per partition.

The 3x3 conv batches 4 images as 9 PSUM-accumulated block-diagonal [128x128]
matmuls over a zero-padded [128, 34, 34] fp16 buffer (images at partition
strips 0/32/64/96). The conv activation emits the whole 128-partition half in
ONE ScalarE op into a strip-stacked s2g buffer; strip row 32i+8 is the
ones-row for folding bu (conv-weight rows there are zero so PSUM is exactly 0,
and its activation bias is v* with silu(v*)=1).

Up-projection uses PE row tiling: contraction K=9 rounds to a 32-row tile, so
the 4 images' matmuls (stationary wu3 replicated at partition strips, moving
s2g strips read in place) land on row groups 0/32/64/96 and execute
CONCURRENTLY in the PE array (~4x effective throughput for this phase -- it
stays off the critical path even at the low HAM p-state). Loop order
(h, cc, i) keeps adjacent matmuls on different row groups; PSUM->SBUF
f32->fp16 copies split 2:1 between VectorE and ScalarE.

Scaling trick: silu(1.702*(v+b)) = 1.702*quick_gelu(v+b), so each activation
is one ScalarE op (scale=1.702, bias=1.702*b, func=Silu); the 1.702 factors
are divided out of the downstream weights (Wc, Wu).
"""

import sys
import numpy as np

for _p in ("/opt/trn_rl_repo",):
    if _p not in sys.path:
        sys.path.append(_p)

import concourse.bacc as bacc
import concourse.mybir as mybir
import concourse.tile as tile
from concourse.bass_utils import run_bass_kernel_spmd

P = 128
N_CORES = 8
B, N, C, DIM = 64, 1024, 768, 8
IPC = B // N_CORES          # images per core
KC = C // P                 # 6 contraction chunks
H = 32                      # patch grid
AF = mybir.ActivationFunctionType
F32 = mybir.dt.float32
F16 = mybir.dt.float16
GS = 1.702
VSTAR = 1.2784645427610737  # silu(VSTAR) == 1.0

# const blob layout (fp16 elements per partition)
O_WD = 0                    # [P, KC*DIM]      48
O_WCBD = O_WD + KC * DIM    # [P, 9*P]         1152
O_WU3R = O_WCBD + 9 * P     # [P, C]           768
O_BCR = O_WU3R + C          # [P, 2] = f32     2
O_BDR = O_BCR + 2           # [8, 2] = f32     2
CBW = O_BDR + 2             # 1972

# input load batching: images per DMA (image 0 is loaded separately in halves)
LOAD_BATCHES = ((1, 1), (2, 2), (4, 2), (6, 2))

_NC_CACHE = None


def _build_nc():
    nc = bacc.Bacc(None, target_bir_lowering=False)

    xt_d = nc.dram_tensor("xt", [P, IPC, KC, N], F16, kind="ExternalInput")
    cb_d = nc.dram_tensor("cb", [P, CBW], F16, kind="ExternalInput")
    out_d = nc.dram_tensor("out", [IPC, 2, 2, P, (KC // 2) * 512], F16,
                           kind="ExternalOutput")

    with tile.TileContext(nc) as tc:
        with (
            tc.tile_pool(name="const", bufs=1) as const,
            tc.tile_pool(name="xt", bufs=4) as xt_pool,
            tc.tile_pool(name="pad", bufs=2) as pad_pool,
            tc.tile_pool(name="s2", bufs=4) as s2_pool,
            tc.tile_pool(name="stag", bufs=6) as stag_pool,
            tc.tile_pool(name="ps_d", bufs=2, space="PSUM") as ps_d,
            tc.tile_pool(name="ps_c", bufs=2, space="PSUM") as ps_c,
            tc.tile_pool(name="ps_u", bufs=4, space="PSUM") as ps_u,
        ):
            # Bulk DMA rides the sync HWDGE ring in strict order (loads, then
            # stores): a single FIFO ring streams back-to-back at HBM rate
            # with no idle. The small const blob goes on the otherwise-idle
            # scalar ring so it doesn't delay image 0.
            cb_s = const.tile([P, CBW], F16)
            nc.scalar.dma_start(cb_s[:], cb_d[:])
            wd_s = cb_s[:, O_WD:O_WCBD].rearrange("p (k d) -> p k d", k=KC)
            wcbd_s = cb_s[:, O_WCBD:O_WU3R].rearrange("p (t c) -> p t c", t=9)
            wu3r_s = cb_s[:, O_WU3R:O_BCR]
            bcr_s = cb_s[:, O_BCR:O_BDR].bitcast(F32)
            bdr_s = cb_s[0:DIM, O_BDR:CBW].bitcast(F32)

            # batched input loads; xts[img] = (tile, slot within tile).
            # Image 0 is loaded as two half-image pieces so the first
            # down-projection starts one half-load sooner.
            batches = list(LOAD_BATCHES)
            xts = {}

            def issue_load():
                if not batches:
                    return
                first, nimg = batches.pop(0)
                t = xt_pool.tile([P, nimg, KC, N], F16,
                                 name=f"xt{first}", tag="xt")
                nc.sync.dma_start(t[:], xt_d[:, first:first + nimg])
                for j in range(nimg):
                    xts[first + j] = (t, j)

            xt0h = []
            for h in range(2):
                t = xt_pool.tile([P, KC, 512], F16, name=f"xt0h{h}", tag="xt")
                nc.sync.dma_start(t[:], xt_d[:, 0, :, h * 512:(h + 1) * 512])
                xt0h.append(t)
            xts[0] = (xt0h, None)
            for _ in range(2):
                issue_load()

            for g in range(IPC // 4):
                padbuf = pad_pool.tile([P, H + 2, H + 2], F16)
                nc.gpsimd.memset(padbuf[:].bitcast(F32), 0.0)

                for i in range(4):
                    img = 4 * g + i
                    xt, j = xts.pop(img)
                    if img == 2:
                        # issue ALL remaining loads now so every load sits
                        # ahead of every store in the shared ring's FIFO
                        issue_load()
                        issue_load()
                    for h in range(2):
                        psd = ps_d.tile([DIM, 512], F32)
                        for k in range(KC):
                            if j is None:       # image 0: per-half tiles
                                moving = xt[h][:, k, :]
                            else:
                                moving = xt[:, j, k, h * 512:(h + 1) * 512]
                            nc.tensor.matmul(
                                psd[:],
                                wd_s[:, k, :],
                                moving,
                                start=(k == 0),
                                stop=(k == KC - 1),
                            )
                        # silu(1.702*(d1 + bd)) -> image strip of padded grid
                        nc.scalar.activation(
                            padbuf[32 * i:32 * i + DIM,
                                   1 + 16 * h:1 + 16 * h + 16, 1:33],
                            psd[:].rearrange("p (a b) -> p a b", a=16),
                            AF.Silu,
                            bias=bdr_s[:],
                            scale=GS,
                        )

                # 3x3 conv, 4 images at once: 9 block-diagonal matmuls per half
                s2gs = []
                for h in range(2):
                    psc = ps_c.tile([P, 512], F32, tag="psc", name=f"psc{h}")
                    for t9 in range(9):
                        dy, dx = t9 // 3, t9 % 3
                        nc.tensor.matmul(
                            psc[:],
                            wcbd_s[:, t9, :],
                            padbuf[:, 16 * h + dy:16 * h + dy + 16, dx:dx + 32],
                            start=(t9 == 0),
                            stop=(t9 == 8),
                        )
                    # one activation for all 4 strips; strip row 32i+8 becomes
                    # the ones-row (PSUM there is exactly 0, silu(VSTAR)=1)
                    s2g = s2_pool.tile([P, 512], F16, tag="s2g", name=f"s2g{h}")
                    s2gs.append(s2g)
                    nc.scalar.activation(
                        s2g[:],
                        psc[:],
                        AF.Silu,
                        bias=bcr_s[:],
                        scale=GS,
                    )

                # up-projection: row-tiled quads -- adjacent matmuls sit on
                # disjoint 32-row PE groups and run concurrently. Half-image
                # staging: each half's 4 stores flush while the other half
                # computes.
                for h in range(2):
                    for q in range(2):      # cc piece: {0,1,2} / {3,4,5}
                        stags = [stag_pool.tile([P, 3 * 512], F16,
                                                name="stag", tag="stag")
                                 for _ in range(4)]
                        for kk in range(3):
                            cc = 3 * q + kk
                            for i in range(4):
                                # last group: ps_d/ps_c banks are idle (no
                                # further downs/convs), so rotate its quads
                                # across all 8 PSUM banks -- two quads in
                                # flight hides the sem-latency that otherwise
                                # paces the kernel tail
                                uq = ((h * 2 + q) * 3 + kk) * 4 + i
                                if g == 1:
                                    r = uq % 8
                                    if r < 4:
                                        psu = ps_u.tile([P, 512], F32)
                                    elif r < 6:
                                        psu = ps_d.tile([P, 512], F32,
                                                        tag="psd",
                                                        name="psu_d")
                                    else:
                                        psu = ps_c.tile([P, 512], F32,
                                                        tag="psc",
                                                        name="psu_c")
                                else:
                                    # group 0: ps_d is live (group 1's downs
                                    # overlap) but the conv banks are idle
                                    # between the two convs -- borrow them
                                    r = uq % 6
                                    if r < 4:
                                        psu = ps_u.tile([P, 512], F32)
                                    else:
                                        psu = ps_c.tile([P, 512], F32,
                                                        tag="psc",
                                                        name="psu_c")
                                # explicit tile_position: the auto-infer path
                                # rejects base partition 96
                                nc.tensor.matmul(
                                    psu[:],
                                    wu3r_s[32 * i:32 * i + DIM + 1,
                                           cc * P:(cc + 1) * P],
                                    s2gs[h][32 * i:32 * i + DIM + 1, :],
                                    start=True,
                                    stop=True,
                                    tile_position=(32 * i, 0),
                                )
                                dst = stags[i][:, kk * 512:(kk + 1) * 512]
                                # group 0: ScalarE also runs group 1's
                                # activations -> bias copies 2:1 to VectorE.
                                # group 1 (kernel tail): nothing else left,
                                # split copies evenly.
                                if g == 0:
                                    on_dve = (cc + h + i) % 3 != 0
                                else:
                                    on_dve = (cc + h + i) % 2 == 0
                                if on_dve:
                                    nc.vector.tensor_copy(dst, psu[:])
                                else:
                                    nc.scalar.copy(dst, psu[:])
                        for i in range(4):
                            img = 4 * g + i
                            # single contiguous 3 KiB line per partition
                            nc.sync.dma_start(out_d[img, h, q], stags[i][:])
    nc.compile()
    return nc


def _get_nc():
    global _NC_CACHE
    if _NC_CACHE is None:
        _NC_CACHE = _build_nc()
    return _NC_CACHE


def kernel(x, Wd, bd, Wc, bc, Wu, bu, _trace=False, _trace_kwargs=None):
    x = np.asarray(x, dtype=np.float32)
    Wd = np.asarray(Wd, dtype=np.float32)
    bd = np.asarray(bd, dtype=np.float32)
    Wc = np.asarray(Wc, dtype=np.float32)
    bc = np.asarray(bc, dtype=np.float32)
    Wu = np.asarray(Wu, dtype=np.float32)
    bu = np.asarray(bu, dtype=np.float32)

    # packed const blob
    wd_h = np.ascontiguousarray(
        Wd.astype(np.float16).reshape(KC, P, DIM).transpose(1, 0, 2)
    ).reshape(P, KC * DIM)
    wcbd_h = np.zeros((P, 9, P), dtype=np.float16)
    for t9 in range(9):
        blk = (Wc[t9 // 3, t9 % 3] / GS).astype(np.float16)     # [ci, co]
        for i in range(4):
            wcbd_h[32 * i:32 * i + DIM, t9, 32 * i:32 * i + DIM] = blk
    wu3_h = np.concatenate(
        [Wu / GS, bu[None, :]], axis=0).astype(np.float16)       # [9, 768]
    wu3r_h = np.zeros((P, C), dtype=np.float16)
    for i in range(4):
        wu3r_h[32 * i:32 * i + DIM + 1] = wu3_h
    bcr_h = np.zeros((P, 1), dtype=np.float32)
    for i in range(4):
        bcr_h[32 * i:32 * i + DIM, 0] = GS * bc
        bcr_h[32 * i + DIM, 0] = VSTAR
    bdr_h = np.zeros((P, 1), dtype=np.float32)
    bdr_h[0:DIM, 0] = GS * bd
    cb_h = np.concatenate([
        wd_h,
        wcbd_h.reshape(P, 9 * P),
        wu3r_h,
        bcr_h.view(np.float16),
        bdr_h.view(np.float16),
    ], axis=1)
    assert cb_h.shape == (P, CBW) and cb_h.dtype == np.float16

    x16 = x.astype(np.float16)                                   # [B, N, C]
    in_maps = []
    for c in range(N_CORES):
        sh = x16[c * IPC:(c + 1) * IPC]                          # [IPC, N, C]
        t = sh.transpose(2, 0, 1)                                # [C, IPC, N]
        xt_h = np.ascontiguousarray(
            t.reshape(KC, P, IPC, N).transpose(1, 2, 0, 3))      # [P,IPC,KC,N]
        in_maps.append({"xt": xt_h, "cb": cb_h})

    nc = _get_nc()
    res = run_bass_kernel_spmd(
        nc, in_maps, core_ids=list(range(N_CORES)),
        trace=_trace, **(_trace_kwargs or {}),
    )
    kernel.last_result = res
    outs = []
    for r in res.results:
        o = r["out"].astype(np.float32)            # [IPC, 2, 2, P, 3*512]
        o = o.reshape(IPC, 2, 2, P, 3, 512)        # [i, h, q, p, kk, n']
        o = o.transpose(0, 1, 5, 2, 4, 3)          # [i, h, n', q, kk, p]
        outs.append(o.reshape(IPC, N, C))
    return np.concatenate(outs, axis=0)
